# revision 1
# baseline (speedup 1.0000x reference)
import numpy as np

# nn_GRSA_23673859735853 — window attention with cosine attention,
# CPB-MLP relative position bias, dual-branch residual projections.
# Shapes: x (2048, 64, 256), mask (64, 64, 64), HEADS=8.
#
# Self-contained full-input -> full-output implementation. The math
# mirrors the reference forward exactly; computed in float32 numpy so
# the result is bit-faithful regardless of the grading environment.


def _l2norm(v, eps=1e-12):
    nrm = np.sqrt(np.sum(v * v, axis=-1, keepdims=True))
    return v / np.maximum(nrm, eps)


def kernel(x, mask, qw, qb, kw, kb, vw, vb, pw, pb,
           mlp_w1, mlp_b1, mlp_w2, logit_scale, rpb_table, rpi):
    x = np.asarray(x, dtype=np.float32)
    mask = np.asarray(mask, dtype=np.float32)
    b, n, c = x.shape
    h = logit_scale.shape[0]
    c2 = c // 2
    hd = c // h
    xb = x.reshape(b, n, 2, c2)

    def dual(w, bias):
        w = np.asarray(w, dtype=np.float32)
        # einsum bnsc,scd->bnsd as two (b*n, c2)@(c2, c2) matmuls
        flat = xb.reshape(b * n, 2, c2)
        out = np.empty_like(flat)
        out[:, 0] = flat[:, 0] @ w[0]
        out[:, 1] = flat[:, 1] @ w[1]
        return xb + out.reshape(b, n, 2, c2) + np.asarray(bias, dtype=np.float32)

    q = dual(qw, qb).reshape(b, n, h, hd).transpose(0, 2, 1, 3)
    k = dual(kw, kb).reshape(b, n, h, hd).transpose(0, 2, 1, 3)
    v = dual(vw, vb).reshape(b, n, h, hd).transpose(0, 2, 1, 3)

    qn = _l2norm(q)
    kn = _l2norm(k)
    attn = np.einsum('bhnd,bhmd->bhnm', qn, kn, optimize=True)
    scale = np.exp(np.minimum(np.asarray(logit_scale, dtype=np.float32),
                              np.log(np.float32(100.0))))  # (h,1,1)
    attn = attn * scale[None]

    # CPB-MLP relative position bias
    tblin = np.asarray(rpb_table, dtype=np.float32)
    hid = np.maximum(tblin @ np.asarray(mlp_w1, dtype=np.float32)
                     + np.asarray(mlp_b1, dtype=np.float32), 0.0)
    tbl = (hid @ np.asarray(mlp_w2, dtype=np.float32)).reshape(-1, h)
    rpi_flat = np.asarray(rpi).reshape(-1).astype(np.int64)
    bias = tbl[rpi_flat].reshape(n, n, h).transpose(2, 0, 1)
    attn = attn + np.float32(16.0) / (1.0 + np.exp(-bias))[None]

    nw = mask.shape[0]
    attn = attn.reshape(b // nw, nw, h, n, n) + mask[None, :, None]
    attn = attn.reshape(b, h, n, n)

    attn = attn - attn.max(axis=-1, keepdims=True)
    np.exp(attn, out=attn)
    attn /= attn.sum(axis=-1, keepdims=True)

    out = np.einsum('bhnm,bhmd->bhnd', attn, v, optimize=True)
    out = out.transpose(0, 2, 1, 3).reshape(b, n, c)

    ob = out.reshape(b * n, 2, c2)
    pw = np.asarray(pw, dtype=np.float32)
    y = np.empty_like(ob)
    y[:, 0] = ob[:, 0] @ pw[0]
    y[:, 1] = ob[:, 1] @ pw[1]
    y = y.reshape(b, n, 2, c2) + np.asarray(pb, dtype=np.float32)
    return y.reshape(b, n, c).astype(np.float32)



# revision 24
# speedup vs baseline: 4.8113x; 4.8113x over previous
import os
import sys

for _p in ('/opt/trn_rl_repo', '/root/.axon_site/_ro/trn_rl_repo'):
    if os.path.isdir(_p) and _p not in sys.path:
        sys.path.insert(0, _p)

import numpy as np
import ml_dtypes

from concourse import bass, bacc, mybir, tile
from concourse.bass_utils import run_bass_kernel_spmd

F32 = mybir.dt.float32
BF16 = mybir.dt.bfloat16
F16 = mybir.dt.float16
AF = mybir.ActivationFunctionType
ALU = mybir.AluOpType

NCORES = 8
B, N, C = 2048, 64, 256     # windows, tokens/window, channels
NW = 64                      # number of masks
H = 8                        # heads
WB = B // NCORES             # 256 windows per core
NBLK = WB // 2               # 128 blocks of 2 windows (128 tokens)

_CACHE = {}
_DEBUG_SKIP_TAIL = False
_DEBUG_STAGE = 5
_DEBUG_QK_HEADS = tuple(range(8))


def _build_program(nblk=NBLK):
    nc = bacc.Bacc(None, target_bir_lowering=False)

    x_d = nc.declare_dram_parameter("x", [nblk, 128, 256], F32, isOutput=False)
    # A addend, compact: [128 = (w:2 x m:64), 32 wi-pairs, 8 heads, 64 n]
    a_d = nc.declare_dram_parameter("abias", [128, 32, 8, 64], F16, isOutput=False)
    # 8 weights: qw0,qw1,kw0,kw1,vw0,vw1,pw0,pw1  (I folded into qkv)
    w_d = nc.declare_dram_parameter("wts", [8, 128, 128], BF16, isOutput=False)
    i_d = nc.declare_dram_parameter("ident", [128, 128], F32, isOutput=False)
    b_d = nc.declare_dram_parameter("bmat", [128, 4], BF16, isOutput=False)
    e_d = nc.declare_dram_parameter("emat", [4, 128], BF16, isOutput=False)
    s_d = nc.declare_dram_parameter("scal", [4, 2], F32, isOutput=False)
    y_d = nc.declare_dram_parameter("y", [nblk, 128, 256], F32, isOutput=True)

    with tile.TileContext(nc) as tc:
        with (
            tc.tile_pool(name="const", bufs=1) as cpool,
            tc.tile_pool(name="xin", bufs=3) as xpool,
            tc.tile_pool(name="work", bufs=2) as wpool,
            tc.tile_pool(name="outp", bufs=2) as opool,
            tc.tile_pool(name="ps_big", bufs=4, space="PSUM") as ps_big,
            tc.tile_pool(name="ps_qk", bufs=1, space="PSUM") as ps_qk,
            tc.tile_pool(name="ps_v", bufs=1, space="PSUM") as ps_v,
            tc.tile_pool(name="ps_av", bufs=2, space="PSUM") as ps_av,
        ):
            # ---- constants resident in SBUF ----
            a_s = cpool.tile([128, 32, H, 64], F16)
            nc.sync.dma_start(a_s[:], a_d[:])
            w_s = cpool.tile([128, 8 * 128], BF16)
            nc.sync.dma_start(
                w_s.rearrange("p (w f) -> p w f", w=8),
                w_d.rearrange("w p f -> p w f"),
            )
            ident = cpool.tile([128, 128], F32)
            nc.sync.dma_start(ident[:], i_d[:])
            bmat = cpool.tile([128, 4], BF16)
            nc.sync.dma_start(bmat[:], b_d[:])
            emat = cpool.tile([4, 128], BF16)
            nc.sync.dma_start(emat[:], e_d[:])
            scal = cpool.tile([4, 2], F32)
            nc.sync.dma_start(scal[:], s_d[:])
            ones = cpool.tile([128, 1], BF16)
            nc.vector.memset(ones[:], 1.0)

            def W(idx):
                return w_s[:, idx * 128:(idx + 1) * 128]

            for i in range(nblk):
                wp = i % 32  # wi = 2*wp + w

                # ---- load x block (128 tokens, 256 ch) ----
                xrow = xpool.tile([128, 256], F32, tag="xrow")
                nc.sync.dma_start(xrow[:], x_d[i])

                # ---- transpose x halves on PE ----
                pxt = ps_big.tile([128, 256], F32, tag="big")
                for s in range(2):
                    nc.tensor.transpose(
                        pxt[:, s * 128:(s + 1) * 128],
                        xrow[:, s * 128:(s + 1) * 128],
                        ident[:],
                    )
                xts = wpool.tile([128, 256], BF16, tag="xts")
                for s in range(2):
                    nc.scalar.activation(
                        xts[:, s * 128:(s + 1) * 128],
                        pxt[:, s * 128:(s + 1) * 128],
                        AF.Copy,
                    )

                # ---- projections ----
                # psq free layout: [qT0 | qT1 | kT0 | kT1] (ch-major)
                psq = ps_qk.tile([128, 512], F32, tag="qk")
                nc.tensor.matmul(psq[:, 0:128], W(0), xts[:, 0:128],
                                 start=True, stop=True)
                nc.tensor.matmul(psq[:, 128:256], W(1), xts[:, 128:256],
                                 start=True, stop=True)
                nc.tensor.matmul(psq[:, 256:384], W(2), xts[:, 0:128],
                                 start=True, stop=True)
                nc.tensor.matmul(psq[:, 384:512], W(3), xts[:, 128:256],
                                 start=True, stop=True)
                # v token-major: halves at free offsets
                psv = ps_v.tile([128, 256], F32, tag="v")
                nc.tensor.matmul(psv[:, 0:128], xts[:, 0:128], W(4),
                                 start=True, stop=True)
                nc.tensor.matmul(psv[:, 128:256], xts[:, 128:256], W(5),
                                 start=True, stop=True)

                if _DEBUG_STAGE <= 1:
                    ybuf = opool.tile([128, 256], F32, tag="ybuf")
                    nc.vector.tensor_copy(ybuf[:], psq[:, 0:256])
                    nc.sync.dma_start(y_d[i], ybuf[:])
                    continue

                # ---- copy q/k to sbuf, squares -> per-head norms ----
                qk16 = wpool.tile([128, 512], BF16, tag="qk16")
                nc.vector.tensor_copy(qk16[:], psq[:])
                sq = wpool.tile([128, 512], BF16, tag="sq")
                nc.scalar.activation(sq[:], qk16[:], AF.Square)
                psn = ps_big.tile([4, 512], F32, tag="big")
                nc.tensor.matmul(psn[:], bmat[:], sq[:], start=True, stop=True)
                srt = wpool.tile([4, 512], F32, tag="srt")
                nc.scalar.activation(srt[:], psn[:], AF.Sqrt)
                rec = wpool.tile([4, 512], F32, tag="rec")
                nc.vector.reciprocal(rec[:], srt[:])
                # fold logit scale into q reciprocals (per-partition scalars)
                nc.vector.tensor_scalar_mul(rec[:, 0:128], rec[:, 0:128],
                                            scal[:, 0:1])
                nc.vector.tensor_scalar_mul(rec[:, 128:256], rec[:, 128:256],
                                            scal[:, 1:2])
                rec16 = wpool.tile([4, 512], BF16, tag="rec16")
                nc.scalar.activation(rec16[:], rec[:], AF.Copy)

                # ---- expand recips across 32-channel groups via PE ----
                S_ps = ps_big.tile([128, 512], F32, tag="big", name=f"S_{i}")
                nc.tensor.matmul(S_ps[:], emat[:], rec16[:],
                                 start=True, stop=True)

                # ---- normalized q|k (ch-major bf16); v to sbuf ----
                qkn = wpool.tile([128, 512], BF16, tag="qkn")
                nc.vector.tensor_tensor(qkn[:], S_ps[:], qk16[:], ALU.mult)
                qn = qkn[:, 0:256]
                kn = qkn[:, 256:512]
                vs = wpool.tile([128, 256], BF16, tag="vs")
                nc.vector.tensor_copy(vs[:], psv[:])

                if _DEBUG_STAGE <= 2:
                    ybuf = opool.tile([128, 256], F32, tag="ybuf")
                    nc.vector.tensor_copy(ybuf[:], qkn[:, 0:256])
                    nc.sync.dma_start(y_d[i], ybuf[:])
                    continue

                # ---- dense qk per head: out (128 k-both-w, 128 q-both-w)
                # head h -> psum bank h%4 (same bank <=> same PE row band,
                # so concurrent drains never share a bank)
                pat = [
                    ps_big.tile([128, 256], F32, tag="big", name=f"pat{j}_{i}")
                    for j in range(4)
                ]
                for j in range(4):
                    for g in range(2):
                        h = j + 4 * g
                        nc.tensor.matmul(
                            pat[j][:, 128 * g:128 * g + 128],
                            kn[32 * j:32 * j + 32, 128 * g:128 * g + 128],
                            qn[32 * j:32 * j + 32, 128 * g:128 * g + 128],
                            start=True, stop=True,
                            tile_position=(32 * j, 0),
                        )

                if _DEBUG_STAGE <= 2.5:
                    ybuf = opool.tile([128, 256], F32, tag="ybuf")
                    nc.vector.tensor_copy(ybuf[:], pat[0][:, 0:256])
                    nc.sync.dma_start(y_d[i], ybuf[:])
                    continue

                # ---- A-add into compact s1 (128, 512): rows 0-63 w0;
                # s1 head order [0,4 | 1,5 | 2,6 | 3,7] (bank-major) ----
                s1 = wpool.tile([128, 512], F32, tag="s1")
                s1r = s1.rearrange("p (g n) -> p g n", n=64)
                for w in range(2):
                    for j in range(4):
                        srcp = pat[j].rearrange("p (g n2) -> p g n2", g=2)
                        nc.vector.tensor_tensor(
                            s1r[64 * w:64 * w + 64, 2 * j:2 * j + 2, :],
                            srcp[64 * w:64 * w + 64, :, 64 * w:64 * w + 64],
                            a_s[64 * w:64 * w + 64, wp, 2 * j:2 * j + 2, :],
                            ALU.add,
                        )

                ex = wpool.tile([128, 512], BF16, tag="ex")
                nc.scalar.activation(ex[:], s1[:], AF.Exp)

                if _DEBUG_STAGE <= 3:
                    ybuf = opool.tile([128, 256], F32, tag="ybuf")
                    nc.vector.tensor_copy(ybuf[:], ex[:, 0:256])
                    nc.sync.dma_start(y_d[i], ybuf[:])
                    continue

                # ---- av: 2-head matmuls + denominator columns ----
                pav = [
                    ps_av.tile([128, 260], F32, tag="av", name=f"pav0_{i}"),
                    ps_av.tile([128, 260], F32, tag="av", name=f"pav1_{i}"),
                ]
                vsr = vs.rearrange("p (g c) -> p g c", g=2)
                for w in range(2):
                    for j in range(4):
                        nc.tensor.matmul(
                            pav[w][:, 64 * j:64 * j + 64],
                            ex[64 * w:64 * w + 64, 128 * j:128 * j + 128],
                            vsr[64 * w:64 * w + 64, :, 32 * j:32 * j + 32],
                            start=True, stop=True,
                            tile_position=(64 * w, 0),
                        )
                        nc.tensor.matmul(
                            pav[w][:, 256 + j:256 + j + 1],
                            ex[64 * w:64 * w + 64, 128 * j:128 * j + 128],
                            ones[64 * w:64 * w + 64, :],
                            start=True, stop=True,
                            tile_position=(64 * w, 0),
                        )

                # ---- denominators -> reciprocals ----
                # rd cols: h2-major pairs: [h0,h1,h2,h3,...] even=lo, odd=hi
                rd = [
                    wpool.tile([64, 8], F32, tag="rd", name=f"rd0_{i}"),
                    wpool.tile([64, 8], F32, tag="rd", name=f"rd1_{i}"),
                ]
                for w in range(2):
                    rdr = rd[w].rearrange("p (n two) -> p n two", two=2)
                    nc.vector.reciprocal(rdr[:, :, 0], pav[w][0:64, 256:260])
                    nc.vector.reciprocal(rdr[:, :, 1], pav[w][64:128, 256:260])

                # ---- divides -> token-major out block (128, 256) bf16 ----
                outblk = opool.tile([128, 256], BF16, tag="outblk")
                obl = outblk[:, 0:128].rearrange("p (j c) -> p j c", j=4)
                obh = outblk[:, 128:256].rearrange("p (j c) -> p j c", j=4)
                for w in range(2):
                    po = pav[w][:, 0:256].rearrange("p (j c) -> p j c", j=4)
                    rdr = rd[w].rearrange("p (n two) -> p n two", two=2)
                    rlo = rdr[:, :, 0:1].broadcast_to((64, 4, 32))
                    rhi = rdr[:, :, 1:2].broadcast_to((64, 4, 32))
                    nc.vector.tensor_tensor(
                        obl[64 * w:64 * w + 64, :, :],
                        po[0:64, :, 0:32], rlo, ALU.mult,
                    )
                    nc.vector.tensor_tensor(
                        obh[64 * w:64 * w + 64, :, :],
                        po[64:128, :, 32:64], rhi, ALU.mult,
                    )

                if _DEBUG_SKIP_TAIL:
                    ybuf = opool.tile([128, 256], F32, tag="ybuf")
                    nc.vector.tensor_copy(ybuf[:], outblk[:])
                    nc.sync.dma_start(y_d[i], ybuf[:])
                else:
                    # ---- transpose out (DMA xbar, bf16) ----
                    outT = opool.tile([128, 256], BF16, tag="outT")
                    for s in range(2):
                        nc.sync.dma_start_transpose(
                            outT[:, 128 * s:128 * s + 128],
                            outblk[:, 128 * s:128 * s + 128],
                        )

                    # ---- output projection -> token-major y ----
                    psy = ps_big.tile([128, 256], F32, tag="big")
                    nc.tensor.matmul(psy[:, 0:128], outT[:, 0:128], W(6),
                                     start=True, stop=True)
                    nc.tensor.matmul(psy[:, 128:256], outT[:, 128:256], W(7),
                                     start=True, stop=True)
                    ybuf = opool.tile([128, 256], F32, tag="ybuf")
                    nc.vector.tensor_copy(ybuf[:], psy[:])
                    nc.sync.dma_start(y_d[i], ybuf[:])

    nc.compile()
    return nc


def _host_prep(inputs):
    x = np.ascontiguousarray(np.asarray(inputs['x'], np.float32))
    mask = np.asarray(inputs['mask'], np.float32)
    qw = np.asarray(inputs['qw'], np.float32)
    kw = np.asarray(inputs['kw'], np.float32)
    vw = np.asarray(inputs['vw'], np.float32)
    pw = np.asarray(inputs['pw'], np.float32)

    # CPB-MLP relative position bias (tiny -> host)
    tbl = np.asarray(inputs['rpb_table'], np.float32)
    hid = np.maximum(
        tbl @ np.asarray(inputs['mlp_w1'], np.float32)
        + np.asarray(inputs['mlp_b1'], np.float32), 0.0)
    t2 = (hid @ np.asarray(inputs['mlp_w2'], np.float32)).reshape(-1, H)
    rpi = np.asarray(inputs['rpi']).reshape(-1).astype(np.int64)
    bias = t2[rpi].reshape(N, N, H)          # (n, m, h)
    b16 = 16.0 / (1.0 + np.exp(-bias))       # 16*sigmoid

    # A compact: [128 = (w:2 x m:64), 32 wi-pairs, 8 heads, 64 n]
    # A[64w+m, p, h, n] = b16[n, m, h] + mask[2p+w, n, m]
    bmh = b16.transpose(1, 2, 0)             # (m, h, n)
    bmh = bmh[:, [0, 4, 1, 5, 2, 6, 3, 7], :]  # bank-major head order
    mT = mask.transpose(0, 2, 1)             # (wi, m, n)
    A = np.empty((2, 64, 32, H, 64), np.float32)
    for w in range(2):
        for p in range(32):
            A[w, :, p, :, :] = bmh + mT[2 * p + w][:, None, :]
    A_dev = np.ascontiguousarray(A.reshape(128, 32, H, 64)).astype(np.float16)

    scales = np.exp(np.minimum(np.asarray(inputs['logit_scale'], np.float32),
                               np.log(np.float32(100.0)))).reshape(H)
    scal = np.ascontiguousarray(
        np.stack([scales[0:4], scales[4:8]], axis=1).astype(np.float32))

    I = np.eye(128, dtype=np.float32)
    wts = np.ascontiguousarray(np.stack([
        I + qw[0], I + qw[1], I + kw[0], I + kw[1],
        I + vw[0], I + vw[1], pw[0], pw[1],
    ]).astype(ml_dtypes.bfloat16))

    bmat = np.zeros((128, 4), np.float32)
    bmat[np.arange(128), np.arange(128) // 32] = 1.0
    emat = np.ascontiguousarray(bmat.T.astype(ml_dtypes.bfloat16))
    bmat = np.ascontiguousarray(bmat.astype(ml_dtypes.bfloat16))

    ident = np.eye(128, dtype=np.float32)

    shards = []
    for c in range(NCORES):
        xs = x[c * WB:(c + 1) * WB].reshape(NBLK, 128, 256)
        shards.append({
            "x": np.ascontiguousarray(xs),
            "abias": A_dev,
            "wts": wts,
            "ident": ident,
            "bmat": bmat,
            "emat": emat,
            "scal": scal,
        })
    return shards


def kernel(**inputs):
    shards = _host_prep(inputs)
    if "nc" not in _CACHE:
        _CACHE["nc"] = _build_program()
    nc = _CACHE["nc"]
    res = run_bass_kernel_spmd(nc, shards, list(range(NCORES)))
    parts = [np.asarray(res.results[c]["y"], np.float32).reshape(WB * N, C)
             for c in range(NCORES)]
    y = np.concatenate(parts, axis=0).reshape(B, N, C)
    return y


# revision 26
# speedup vs baseline: 1048.6149x; 217.9468x over previous
import os
import sys

for _p in ('/opt/trn_rl_repo', '/root/.axon_site/_ro/trn_rl_repo'):
    if os.path.isdir(_p) and _p not in sys.path:
        sys.path.insert(0, _p)

import numpy as np
import ml_dtypes

from concourse import bass, bacc, mybir, tile
from concourse.bass_utils import run_bass_kernel_spmd

F32 = mybir.dt.float32
BF16 = mybir.dt.bfloat16
F16 = mybir.dt.float16
AF = mybir.ActivationFunctionType
ALU = mybir.AluOpType

NCORES = 8
B, N, C = 2048, 64, 256     # windows, tokens/window, channels
NW = 64                      # number of masks
H = 8                        # heads
WB = B // NCORES             # 256 windows per core
NBLK = WB // 2               # 128 blocks of 2 windows (128 tokens)

_CACHE = {}
_DEBUG_SKIP_TAIL = False
_DEBUG_STAGE = 5
_DEBUG_QK_HEADS = tuple(range(8))


def _build_program(nblk=NBLK):
    nc = bacc.Bacc(None, target_bir_lowering=False)

    x_d = nc.declare_dram_parameter("x", [nblk, 128, 256], F32, isOutput=False)
    # A addend, compact: [128 = (w:2 x m:64), 32 wi-pairs, 8 heads, 64 n]
    a_d = nc.declare_dram_parameter("abias", [128, 32, 8, 64], F16, isOutput=False)
    # 8 weights: qw0,qw1,kw0,kw1,vw0,vw1,pw0,pw1  (I folded into qkv)
    w_d = nc.declare_dram_parameter("wts", [8, 128, 128], BF16, isOutput=False)
    i_d = nc.declare_dram_parameter("ident", [128, 128], F32, isOutput=False)
    b_d = nc.declare_dram_parameter("bmat", [128, 4], BF16, isOutput=False)
    e_d = nc.declare_dram_parameter("emat", [4, 128], BF16, isOutput=False)
    s_d = nc.declare_dram_parameter("scal", [4, 2], F32, isOutput=False)
    y_d = nc.declare_dram_parameter("y", [nblk, 128, 256], F32, isOutput=True)

    with tile.TileContext(nc) as tc:
        with (
            tc.tile_pool(name="const", bufs=1) as cpool,
            tc.tile_pool(name="xin", bufs=3) as xpool,
            tc.tile_pool(name="work", bufs=2) as wpool,
            tc.tile_pool(name="outp", bufs=2) as opool,
            tc.tile_pool(name="ps_big", bufs=4, space="PSUM") as ps_big,
            tc.tile_pool(name="ps_qk", bufs=1, space="PSUM") as ps_qk,
            tc.tile_pool(name="ps_v", bufs=1, space="PSUM") as ps_v,
            tc.tile_pool(name="ps_av", bufs=2, space="PSUM") as ps_av,
        ):
            # ---- constants resident in SBUF ----
            a_s = cpool.tile([128, 32, H, 64], F16)
            nc.sync.dma_start(a_s[:], a_d[:])
            w_s = cpool.tile([128, 8 * 128], BF16)
            nc.sync.dma_start(
                w_s.rearrange("p (w f) -> p w f", w=8),
                w_d.rearrange("w p f -> p w f"),
            )
            ident = cpool.tile([128, 128], F32)
            nc.sync.dma_start(ident[:], i_d[:])
            bmat = cpool.tile([128, 4], BF16)
            nc.sync.dma_start(bmat[:], b_d[:])
            emat = cpool.tile([4, 128], BF16)
            nc.sync.dma_start(emat[:], e_d[:])
            scal = cpool.tile([4, 2], F32)
            nc.sync.dma_start(scal[:], s_d[:])
            ones = cpool.tile([128, 1], BF16)
            nc.vector.memset(ones[:], 1.0)

            def W(idx):
                return w_s[:, idx * 128:(idx + 1) * 128]

            for i in range(nblk):
                wp = i % 32  # wi = 2*wp + w

                # ---- load x block (128 tokens, 256 ch) ----
                xrow = xpool.tile([128, 256], F32, tag="xrow")
                nc.sync.dma_start(xrow[:], x_d[i])

                # ---- transpose x halves on PE ----
                pxt = ps_big.tile([128, 256], F32, tag="big")
                for s in range(2):
                    nc.tensor.transpose(
                        pxt[:, s * 128:(s + 1) * 128],
                        xrow[:, s * 128:(s + 1) * 128],
                        ident[:],
                    )
                xts = wpool.tile([128, 256], BF16, tag="xts")
                for s in range(2):
                    nc.scalar.activation(
                        xts[:, s * 128:(s + 1) * 128],
                        pxt[:, s * 128:(s + 1) * 128],
                        AF.Copy,
                    )

                # ---- projections ----
                # psq free layout: [qT0 | qT1 | kT0 | kT1] (ch-major)
                psq = ps_qk.tile([128, 512], F32, tag="qk")
                nc.tensor.matmul(psq[:, 0:128], W(0), xts[:, 0:128],
                                 start=True, stop=True)
                nc.tensor.matmul(psq[:, 128:256], W(1), xts[:, 128:256],
                                 start=True, stop=True)
                nc.tensor.matmul(psq[:, 256:384], W(2), xts[:, 0:128],
                                 start=True, stop=True)
                nc.tensor.matmul(psq[:, 384:512], W(3), xts[:, 128:256],
                                 start=True, stop=True)
                # v token-major: halves at free offsets
                psv = ps_v.tile([128, 256], F32, tag="v")
                nc.tensor.matmul(psv[:, 0:128], xts[:, 0:128], W(4),
                                 start=True, stop=True)
                nc.tensor.matmul(psv[:, 128:256], xts[:, 128:256], W(5),
                                 start=True, stop=True)

                if _DEBUG_STAGE <= 1:
                    ybuf = opool.tile([128, 256], F32, tag="ybuf")
                    nc.vector.tensor_copy(ybuf[:], psq[:, 0:256])
                    nc.sync.dma_start(y_d[i], ybuf[:])
                    continue

                # ---- copy q/k to sbuf, squares -> per-head norms ----
                qk16 = wpool.tile([128, 512], BF16, tag="qk16")
                nc.vector.tensor_copy(qk16[:], psq[:])
                sq = wpool.tile([128, 512], BF16, tag="sq")
                nc.scalar.activation(sq[:], qk16[:], AF.Square)
                psn = ps_big.tile([4, 512], F32, tag="big")
                nc.tensor.matmul(psn[:], bmat[:], sq[:], start=True, stop=True)
                srt = wpool.tile([4, 512], F32, tag="srt")
                nc.scalar.activation(srt[:], psn[:], AF.Sqrt)
                rec = wpool.tile([4, 512], F32, tag="rec")
                nc.vector.reciprocal(rec[:], srt[:])
                # fold logit scale into q reciprocals (per-partition scalars)
                nc.vector.tensor_scalar_mul(rec[:, 0:128], rec[:, 0:128],
                                            scal[:, 0:1])
                nc.vector.tensor_scalar_mul(rec[:, 128:256], rec[:, 128:256],
                                            scal[:, 1:2])
                rec16 = wpool.tile([4, 512], BF16, tag="rec16")
                nc.scalar.activation(rec16[:], rec[:], AF.Copy)

                # ---- expand recips across 32-channel groups via PE ----
                S_ps = ps_big.tile([128, 512], F32, tag="big", name=f"S_{i}")
                nc.tensor.matmul(S_ps[:], emat[:], rec16[:],
                                 start=True, stop=True)

                # ---- normalized q|k (ch-major bf16); v to sbuf ----
                qkn = wpool.tile([128, 512], BF16, tag="qkn")
                nc.vector.tensor_tensor(qkn[:], S_ps[:], qk16[:], ALU.mult)
                qn = qkn[:, 0:256]
                kn = qkn[:, 256:512]
                vs = wpool.tile([128, 256], BF16, tag="vs")
                nc.vector.tensor_copy(vs[:], psv[:])

                if _DEBUG_STAGE <= 2:
                    ybuf = opool.tile([128, 256], F32, tag="ybuf")
                    nc.vector.tensor_copy(ybuf[:], qkn[:, 0:256])
                    nc.sync.dma_start(y_d[i], ybuf[:])
                    continue

                # ---- dense qk per head: out (128 k-both-w, 128 q-both-w)
                # head h -> psum bank h%4 (same bank <=> same PE row band,
                # so concurrent drains never share a bank)
                pat = [
                    ps_big.tile([128, 256], F32, tag="big", name=f"pat{j}_{i}")
                    for j in range(4)
                ]
                for j in range(4):
                    for g in range(2):
                        h = j + 4 * g
                        nc.tensor.matmul(
                            pat[j][:, 128 * g:128 * g + 128],
                            kn[32 * j:32 * j + 32, 128 * g:128 * g + 128],
                            qn[32 * j:32 * j + 32, 128 * g:128 * g + 128],
                            start=True, stop=True,
                            tile_position=(32 * j, 0),
                        )

                if _DEBUG_STAGE <= 2.5:
                    ybuf = opool.tile([128, 256], F32, tag="ybuf")
                    nc.vector.tensor_copy(ybuf[:], pat[0][:, 0:256])
                    nc.sync.dma_start(y_d[i], ybuf[:])
                    continue

                # ---- A-add into compact s1 (128, 512): rows 0-63 w0;
                # s1 head order [0,4 | 1,5 | 2,6 | 3,7] (bank-major) ----
                s1 = wpool.tile([128, 512], F32, tag="s1")
                s1r = s1.rearrange("p (g n) -> p g n", n=64)
                for w in range(2):
                    for j in range(4):
                        srcp = pat[j].rearrange("p (g n2) -> p g n2", g=2)
                        nc.vector.tensor_tensor(
                            s1r[64 * w:64 * w + 64, 2 * j:2 * j + 2, :],
                            srcp[64 * w:64 * w + 64, :, 64 * w:64 * w + 64],
                            a_s[64 * w:64 * w + 64, wp, 2 * j:2 * j + 2, :],
                            ALU.add,
                        )

                ex = wpool.tile([128, 512], BF16, tag="ex")
                nc.scalar.activation(ex[:], s1[:], AF.Exp)

                if _DEBUG_STAGE <= 3:
                    ybuf = opool.tile([128, 256], F32, tag="ybuf")
                    nc.vector.tensor_copy(ybuf[:], ex[:, 0:256])
                    nc.sync.dma_start(y_d[i], ybuf[:])
                    continue

                # ---- av: 2-head matmuls + denominator columns ----
                pav = [
                    ps_av.tile([128, 260], F32, tag="av", name=f"pav0_{i}"),
                    ps_av.tile([128, 260], F32, tag="av", name=f"pav1_{i}"),
                ]
                vsr = vs.rearrange("p (g c) -> p g c", g=2)
                for w in range(2):
                    for j in range(4):
                        nc.tensor.matmul(
                            pav[w][:, 64 * j:64 * j + 64],
                            ex[64 * w:64 * w + 64, 128 * j:128 * j + 128],
                            vsr[64 * w:64 * w + 64, :, 32 * j:32 * j + 32],
                            start=True, stop=True,
                            tile_position=(64 * w, 0),
                        )
                        nc.tensor.matmul(
                            pav[w][:, 256 + j:256 + j + 1],
                            ex[64 * w:64 * w + 64, 128 * j:128 * j + 128],
                            ones[64 * w:64 * w + 64, :],
                            start=True, stop=True,
                            tile_position=(64 * w, 0),
                        )

                # ---- denominators -> reciprocals ----
                # rd cols: h2-major pairs: [h0,h1,h2,h3,...] even=lo, odd=hi
                rd = [
                    wpool.tile([64, 8], F32, tag="rd", name=f"rd0_{i}"),
                    wpool.tile([64, 8], F32, tag="rd", name=f"rd1_{i}"),
                ]
                for w in range(2):
                    rdr = rd[w].rearrange("p (n two) -> p n two", two=2)
                    nc.vector.reciprocal(rdr[:, :, 0], pav[w][0:64, 256:260])
                    nc.vector.reciprocal(rdr[:, :, 1], pav[w][64:128, 256:260])

                # ---- divides -> token-major out block (128, 256) bf16 ----
                outblk = opool.tile([128, 256], BF16, tag="outblk")
                obl = outblk[:, 0:128].rearrange("p (j c) -> p j c", j=4)
                obh = outblk[:, 128:256].rearrange("p (j c) -> p j c", j=4)
                for w in range(2):
                    po = pav[w][:, 0:256].rearrange("p (j c) -> p j c", j=4)
                    rdr = rd[w].rearrange("p (n two) -> p n two", two=2)
                    rlo = rdr[:, :, 0:1].broadcast_to((64, 4, 32))
                    rhi = rdr[:, :, 1:2].broadcast_to((64, 4, 32))
                    nc.vector.tensor_tensor(
                        obl[64 * w:64 * w + 64, :, :],
                        po[0:64, :, 0:32], rlo, ALU.mult,
                    )
                    nc.vector.tensor_tensor(
                        obh[64 * w:64 * w + 64, :, :],
                        po[64:128, :, 32:64], rhi, ALU.mult,
                    )

                if _DEBUG_SKIP_TAIL:
                    ybuf = opool.tile([128, 256], F32, tag="ybuf")
                    nc.vector.tensor_copy(ybuf[:], outblk[:])
                    nc.sync.dma_start(y_d[i], ybuf[:])
                else:
                    # ---- transpose out (DMA xbar, bf16) ----
                    outT = opool.tile([128, 256], BF16, tag="outT")
                    for s in range(2):
                        nc.sync.dma_start_transpose(
                            outT[:, 128 * s:128 * s + 128],
                            outblk[:, 128 * s:128 * s + 128],
                        )

                    # ---- output projection -> token-major y ----
                    psy = ps_big.tile([128, 256], F32, tag="big")
                    nc.tensor.matmul(psy[:, 0:128], outT[:, 0:128], W(6),
                                     start=True, stop=True)
                    nc.tensor.matmul(psy[:, 128:256], outT[:, 128:256], W(7),
                                     start=True, stop=True)
                    ybuf = opool.tile([128, 256], F32, tag="ybuf")
                    nc.vector.tensor_copy(ybuf[:], psy[:])
                    nc.sync.dma_start(y_d[i], ybuf[:])

    nc.compile()
    return nc


def _host_prep(inputs):
    x = np.ascontiguousarray(np.asarray(inputs['x'], np.float32))
    mask = np.asarray(inputs['mask'], np.float32)
    qw = np.asarray(inputs['qw'], np.float32)
    kw = np.asarray(inputs['kw'], np.float32)
    vw = np.asarray(inputs['vw'], np.float32)
    pw = np.asarray(inputs['pw'], np.float32)

    # CPB-MLP relative position bias (tiny -> host)
    tbl = np.asarray(inputs['rpb_table'], np.float32)
    hid = np.maximum(
        tbl @ np.asarray(inputs['mlp_w1'], np.float32)
        + np.asarray(inputs['mlp_b1'], np.float32), 0.0)
    t2 = (hid @ np.asarray(inputs['mlp_w2'], np.float32)).reshape(-1, H)
    rpi = np.asarray(inputs['rpi']).reshape(-1).astype(np.int64)
    bias = t2[rpi].reshape(N, N, H)          # (n, m, h)
    b16 = 16.0 / (1.0 + np.exp(-bias))       # 16*sigmoid

    # A compact: [128 = (w:2 x m:64), 32 wi-pairs, 8 heads, 64 n]
    # A[64w+m, p, h, n] = b16[n, m, h] + mask[2p+w, n, m]
    bmh = b16.transpose(1, 2, 0)             # (m, h, n)
    bmh = bmh[:, [0, 4, 1, 5, 2, 6, 3, 7], :]  # bank-major head order
    mT = mask.transpose(0, 2, 1)             # (wi, m, n)
    A = np.empty((2, 64, 32, H, 64), np.float32)
    for w in range(2):
        for p in range(32):
            A[w, :, p, :, :] = bmh + mT[2 * p + w][:, None, :]
    A_dev = np.ascontiguousarray(A.reshape(128, 32, H, 64)).astype(np.float16)

    scales = np.exp(np.minimum(np.asarray(inputs['logit_scale'], np.float32),
                               np.log(np.float32(100.0)))).reshape(H)
    scal = np.ascontiguousarray(
        np.stack([scales[0:4], scales[4:8]], axis=1).astype(np.float32))

    I = np.eye(128, dtype=np.float32)
    wts = np.ascontiguousarray(np.stack([
        I + qw[0], I + qw[1], I + kw[0], I + kw[1],
        I + vw[0], I + vw[1], pw[0], pw[1],
    ]).astype(ml_dtypes.bfloat16))

    bmat = np.zeros((128, 4), np.float32)
    bmat[np.arange(128), np.arange(128) // 32] = 1.0
    emat = np.ascontiguousarray(bmat.T.astype(ml_dtypes.bfloat16))
    bmat = np.ascontiguousarray(bmat.astype(ml_dtypes.bfloat16))

    ident = np.eye(128, dtype=np.float32)

    shards = []
    for c in range(NCORES):
        xs = x[c * WB:(c + 1) * WB].reshape(NBLK, 128, 256)
        shards.append({
            "x": np.ascontiguousarray(xs),
            "abias": A_dev,
            "wts": wts,
            "ident": ident,
            "bmat": bmat,
            "emat": emat,
            "scal": scal,
        })
    return shards


def _get_runner():
    """Build (once) a persistent jitted shard_map executor for the bass
    program, so repeat calls skip retracing and the NEFF compile."""
    if "runner" in _CACHE:
        return _CACHE["runner"]
    import jax
    from jax.experimental.shard_map import shard_map
    from jax.sharding import Mesh, PartitionSpec
    from concourse import mybir as _mb
    from concourse.bass2jax import (_bass_exec_p, install_neuronx_cc_hook,
                                    partition_id_tensor)

    install_neuronx_cc_hook()
    nc = _CACHE.get("nc")
    if nc is None:
        nc = _CACHE["nc"] = _build_program()
    assert nc.dbg_addr is None
    pid_name = (nc.partition_id_tensor.name
                if nc.partition_id_tensor is not None else None)

    in_names, out_names, out_avals = [], [], []
    for alloc in nc.m.functions[0].allocations:
        if not isinstance(alloc, _mb.MemoryLocationSet):
            continue
        name = alloc.memorylocations[0].name
        if alloc.kind == "ExternalInput":
            if name != pid_name:
                in_names.append(name)
        elif alloc.kind == "ExternalOutput":
            out_names.append(name)
            shape = tuple(alloc.tensor_shape)
            out_avals.append(
                jax.core.ShapedArray(shape, _mb.dt.np(alloc.dtype)))
    n_params = len(in_names)
    all_names = in_names + out_names
    if pid_name is not None:
        all_names = all_names + [pid_name]

    def _body(*args):
        operands = list(args)
        if pid_name is not None:
            operands.append(partition_id_tensor())
        outs = _bass_exec_p.bind(
            *operands,
            out_avals=tuple(out_avals),
            in_names=tuple(all_names),
            out_names=tuple(out_names),
            lowering_input_output_aliases=(),
            sim_require_finite=True,
            sim_require_nnan=True,
            nc=nc,
        )
        return tuple(outs)

    devices = jax.devices()[:NCORES]
    mesh = Mesh(np.asarray(devices), ("core",))
    n_out = len(out_names)
    fn = jax.jit(
        shard_map(
            _body, mesh=mesh,
            in_specs=(PartitionSpec("core"),) * (n_params + n_out),
            out_specs=(PartitionSpec("core"),) * n_out,
            check_rep=False,
        ),
        keep_unused=True,
    )
    _CACHE["runner"] = (fn, in_names, out_names, out_avals, mesh)
    return _CACHE["runner"]


def _place_inputs(shards):
    import jax
    from jax.sharding import NamedSharding, PartitionSpec
    fn, in_names, out_names, out_avals, mesh = _get_runner()
    sh = NamedSharding(mesh, PartitionSpec("core"))
    args = []
    for name in in_names:
        cat = np.concatenate([np.asarray(s[name]) for s in shards], axis=0)
        args.append(jax.device_put(cat, sh))
    for av in out_avals:
        z = np.zeros((NCORES * av.shape[0],) + av.shape[1:], av.dtype)
        args.append(jax.device_put(z, sh))
    for a in args:
        a.block_until_ready()
    return args


def _execute(args):
    fn = _get_runner()[0]
    outs = fn(*args)
    for o in outs:
        o.block_until_ready()
    return outs


def kernel(**inputs):
    shards = _host_prep(inputs)
    args = _place_inputs(shards)
    outs = _execute(args)
    y = np.asarray(outs[0], np.float32).reshape(B, N, C)
    return y
